# revision 4
# baseline (speedup 1.0000x reference)
"""MoE gate kernel v2 for Trainium2 (8 NeuronCores, data-parallel over tokens).

Computation per token t (64 experts, top-8):
    gate[t, e]  = sum_h x[t, h] * W[e, h]          (f32-accurate)
    biased      = gate + expert_bias
    top8 of biased -> idx (jax top_k tie semantics)
    weights     = sigmoid(gate[t, idx]) / sum(...)

Precision: hi/lo fp16 split (lo pre-scaled 2^11), as the v1 kernel:
    gate = xh@Wh + 2^-11 * (xh@Wl_s + xl_s@Wh)     (~1e-6 abs err)

v2 layout: x is the STATIONARY operand ([128h x 128t] tiles, FWL-loadable
fp16, dual weight buffers hide the load), W the MOVING operand.  PSUM
comes out [token, expert] directly: no PE transposes, no PSUM->SBUF
transpose copies.  Pass A streams packed [Wh_k | Wl_s_k] (128 cols) over
xh tiles; pass B streams Wh_k (64 cols) over xl tiles accumulating onto
the lo columns.  PE cost 12288 cycles/group vs 16384+transposes in v1.

Top-8 runs on DVE in [t, e] layout: max8/max_index on biased, is_ge mask,
second max8/max_index on masked probs, K x K equality match to reorder
probs into biased-rank order, then L1 normalize.
"""

import numpy as np

N_CORES = 8
H = 2048          # hidden dim = contraction
E = 64            # experts
K = 8             # top-k
T_TOTAL = 16384   # 4*4096 tokens
T_CORE = T_TOTAL // N_CORES   # 2048
NG = 4            # token groups per core
GT = T_CORE // NG             # 512 tokens per group (one PSUM bank of f32)
NT = GT // 128                # 4 token-tiles per group
KC = H // 128                 # 16 contraction chunks
LO_SCALE = float(2.0 ** 11)
INV_LO_SCALE = float(2.0 ** -11)

_CACHE = {}


def _build_nc(repeat=1, mode="full", cpt=4, serialize=False,
              store_mode="group", lo8=True):
    from contextlib import ExitStack

    import concourse.bass as bass  # noqa: F401
    import concourse.tile as tile
    from concourse import bacc, mybir

    f16 = mybir.dt.float16
    f32 = mybir.dt.float32
    u32 = mybir.dt.uint32
    f8 = mybir.dt.float8e4
    xl_dt = f8 if lo8 else f16
    Alu = mybir.AluOpType
    Act = mybir.ActivationFunctionType

    nc = bacc.Bacc(
        "TRN2", target_bir_lowering=False, debug=False, num_devices=N_CORES
    )

    n_x = KC // cpt               # x tiles per group per stream
    rows = cpt // 4               # dram rows per partition per tile

    # DRAM I/O (per core). x shards pre-packed on host (see _host_prep).
    xht_d = nc.dram_tensor("xht", [H, T_CORE], f16, kind="ExternalInput").ap()
    xlt_d = nc.dram_tensor("xlt", [H, T_CORE], xl_dt,
                           kind="ExternalInput").ap()
    whl_d = nc.dram_tensor("whl", [128, KC * 2 * E], f16,
                           kind="ExternalInput").ap()
    if lo8:
        wh8_d = nc.dram_tensor("wh8", [128, KC * E], f8,
                               kind="ExternalInput").ap()
    biasb_d = nc.dram_tensor("biasb", [128, NT * E], f32,
                             kind="ExternalInput").ap()
    oidx_d = nc.dram_tensor("out_idx", [128, NG * NT * K], mybir.dt.int32,
                            kind="ExternalOutput").ap()
    ow_d = nc.dram_tensor("out_w", [128, NG * NT * K], f32,
                          kind="ExternalOutput").ap()
    if mode == "debug":
        dbg_d = nc.dram_tensor("dbg", [128, NG * 3 * NT * 128], f32,
                               kind="ExternalOutput").ap()

    with tile.TileContext(nc) as tc, ExitStack() as ctx:
        xpool = ctx.enter_context(tc.tile_pool(name="x", bufs=1))
        wpool = ctx.enter_context(tc.tile_pool(name="w", bufs=1))
        gpool = ctx.enter_context(tc.tile_pool(name="g", bufs=2))
        ppool = ctx.enter_context(tc.tile_pool(name="mm", bufs=2, space="PSUM"))
        spool = ctx.enter_context(tc.tile_pool(name="s", bufs=2))
        stpool = ctx.enter_context(tc.tile_pool(name="st", bufs=1))

        # weights / bias on the scalar(Act) queue so the sync queue starts
        # streaming xh immediately
        whl = wpool.tile([128, KC * 2 * E], f16, tag="whl")
        nc.scalar.dma_start(whl[:], whl_d)
        biasb = wpool.tile([128, NT * E], f32, tag="biasb")
        nc.scalar.dma_start(biasb[:], biasb_d)
        if lo8:
            wh8 = wpool.tile([128, KC * E], f8, tag="wh8")
            nc.scalar.dma_start(wh8[:], wh8_d)

        idx_st = stpool.tile([128, NG * NT * K], u32, tag="idxst")
        w_st = stpool.tile([128, NG * NT * K], f32, tag="wst")

        if rows == 1:
            src_h = xht_d.rearrange("(b p) f -> b p f", p=128)
            src_l = xlt_d.rearrange("(b p) f -> b p f", p=128)
        else:
            src_h = xht_d.rearrange("(b p r) f -> b p (r f)", p=128, r=rows)
            src_l = xlt_d.rearrange("(b p r) f -> b p (r f)", p=128, r=rows)

        for _rep in range(repeat):
            # ---- x loads, group-major, split across both HWDGE rings ----
            xh_t = [[None] * n_x for _ in range(NG)]
            xl_t = [[None] * n_x for _ in range(NG)]
            for g in range(NG):
                for i in range(n_x):
                    b = g * n_x + i
                    th = xpool.tile([128, cpt * GT], f16, tag=f"xh{g}_{i}")
                    tl = xpool.tile([128, cpt * GT], xl_dt, tag=f"xl{g}_{i}")
                    if serialize and g == 0 and i == 0:
                        # force rep boundary: first x writes on BOTH rings
                        # wait on the previous rep's last top-8 result
                        nc.sync.dma_start(th[0:1, 0:16],
                                          idx_st[0:1, 120:128].bitcast(f16))
                        wsrc = (w_st[0:1, 124:128].bitcast(f8) if lo8
                                else w_st[0:1, 120:128].bitcast(f16))
                        nc.scalar.dma_start(tl[0:1, 0:16], wsrc)
                    nc.sync.dma_start(th[:], src_h[b])
                    nc.scalar.dma_start(tl[:], src_l[b])
                    xh_t[g][i] = th
                    xl_t[g][i] = tl

            def _xs(xt, g, k, j):
                return xt[g][k // cpt][
                    :, (k % cpt) * GT + j * 128:(k % cpt) * GT + (j + 1) * 128]

            for g in range(NG if mode != "dma" else 0):
                # ---- matmuls: psum[t, j*128 + (hi64|lo64)] per tile j ----
                ps = ppool.tile([128, NT * 128], f32, tag="ps")
                # NOTE: start=True zeroes the whole PSUM bank, so exactly one
                # start (first matmul) / stop (last matmul) per bank.
                for k in range(KC):
                    for j in range(NT):
                        nc.tensor.matmul(
                            ps[:, j * 128:(j + 1) * 128],
                            lhsT=_xs(xh_t, g, k, j),
                            rhs=whl[:, k * 2 * E:(k + 1) * 2 * E],
                            start=(k == 0 and j == 0), stop=False)
                for k in range(KC):
                    for j in range(NT):
                        rhs_b = (wh8[:, k * E:(k + 1) * E] if lo8
                                 else whl[:, k * 2 * E:k * 2 * E + E])
                        nc.tensor.matmul(
                            ps[:, j * 128 + E:(j + 1) * 128],
                            lhsT=_xs(xl_t, g, k, j),
                            rhs=rhs_b,
                            start=False,
                            stop=(k == KC - 1 and j == NT - 1))
                if mode == "mm":
                    continue

                psv = ps[:].rearrange("p (j two e) -> p j two e", two=2, e=E)
                if mode == "debug":
                    psb = gpool.tile([128, NT * 128], f32, tag="psb")
                    nc.vector.tensor_copy(psb[:], ps[:])
                    nc.sync.dma_start(
                        dbg_d[:, (g * 3 + 2) * NT * 128:(g * 3 + 3) * NT * 128],
                        psb[:])
                # ---- combine + bias + sigmoid ([t, e] layout) ----
                comb = gpool.tile([128, NT * E], f32, tag="comb")
                combv = comb[:].rearrange("p (j e) -> p j e", e=E)
                nc.scalar.activation(combv, psv[:, :, 1, :], Act.Identity,
                                     scale=INV_LO_SCALE)
                gate = gpool.tile([128, NT * E], f32, tag="gate")
                nc.vector.tensor_tensor(
                    gate[:].rearrange("p (j e) -> p j e", e=E),
                    psv[:, :, 0, :], combv, op=Alu.add)
                probs = gpool.tile([128, NT * E], f32, tag="probs")
                nc.scalar.activation(probs[:], gate[:], Act.Sigmoid)
                biased = gpool.tile([128, NT * E], f32, tag="biased")
                nc.vector.tensor_tensor(biased[:], gate[:], biasb[:],
                                        op=Alu.add)
                if mode == "debug":
                    nc.sync.dma_start(
                        dbg_d[:, (g * 3) * NT * 128:(g * 3) * NT * 128 + NT * E],
                        gate[:])
                    nc.sync.dma_start(
                        dbg_d[:, (g * 3) * NT * 128 + NT * E:
                              (g * 3 + 1) * NT * 128],
                        biased[:])
                    nc.sync.dma_start(
                        dbg_d[:, (g * 3 + 1) * NT * 128:
                              (g * 3 + 1) * NT * 128 + NT * E],
                        probs[:])

                # ---- top-8 (per 128-token tile j; batched where possible) ----
                b8g = spool.tile([128, NT * K], f32, tag="b8g")
                for j in range(NT):
                    nc.vector.max(b8g[:, j * K:(j + 1) * K],
                                  biased[:, j * E:(j + 1) * E])
                    nc.vector.max_index(
                        idx_st[:, (g * NT + j) * K:(g * NT + j + 1) * K],
                        b8g[:, j * K:(j + 1) * K],
                        biased[:, j * E:(j + 1) * E])
                mask = spool.tile([128, NT * E], f32, tag="mask")
                nc.vector.tensor_tensor(
                    mask[:].rearrange("p (t e) -> p t e", e=E),
                    biased[:].rearrange("p (t e) -> p t e", e=E),
                    b8g[:].rearrange("p (t k) -> p t k", k=K)[:, :, 7:8]
                    .broadcast_to((128, NT, E)),
                    op=Alu.is_ge)
                pmask = spool.tile([128, NT * E], f32, tag="pmask")
                nc.vector.tensor_tensor(pmask[:], probs[:], mask[:],
                                        op=Alu.mult)
                p8g = spool.tile([128, NT * K], f32, tag="p8g")
                pidxg = spool.tile([128, NT * K], u32, tag="pidxg")
                for j in range(NT):
                    nc.vector.max(p8g[:, j * K:(j + 1) * K],
                                  pmask[:, j * E:(j + 1) * E])
                    nc.vector.max_index(pidxg[:, j * K:(j + 1) * K],
                                        p8g[:, j * K:(j + 1) * K],
                                        pmask[:, j * E:(j + 1) * E])
                # permute p8 into biased-rank order:
                # w8[t, a] = sum_b p8[t, b] * (pidx[t, b] == bidx[t, a])
                bidx = idx_st[:, g * NT * K:(g + 1) * NT * K]
                eq = spool.tile([128, NT * K * K], f32, tag="eq")
                nc.vector.tensor_tensor(
                    eq[:].rearrange("p (t a b) -> p t a b", a=K, b=K),
                    bidx.rearrange("p (t a) -> p t a", a=K).unsqueeze(3)
                    .broadcast_to((128, NT, K, K)),
                    pidxg[:].rearrange("p (t b) -> p t b", b=K).unsqueeze(2)
                    .broadcast_to((128, NT, K, K)),
                    op=Alu.is_equal)
                wmat = spool.tile([128, NT * K * K], f32, tag="wmat")
                nc.vector.tensor_tensor(
                    wmat[:].rearrange("p (t a b) -> p t a b", a=K, b=K),
                    eq[:].rearrange("p (t a b) -> p t a b", a=K, b=K),
                    p8g[:].rearrange("p (t b) -> p t b", b=K).unsqueeze(2)
                    .broadcast_to((128, NT, K, K)),
                    op=Alu.mult)
                w8g = spool.tile([128, NT * K], f32, tag="w8g")
                nc.vector.tensor_reduce(
                    w8g[:],
                    wmat[:].rearrange("p (ta b) -> p ta b", b=K),
                    axis=mybir.AxisListType.X, op=Alu.add)
                deng = spool.tile([128, NT], f32, tag="deng")
                nc.vector.tensor_reduce(
                    deng[:], w8g[:].rearrange("p (t k) -> p t k", k=K),
                    axis=mybir.AxisListType.X, op=Alu.add)
                recg = spool.tile([128, NT], f32, tag="recg")
                nc.vector.reciprocal(recg[:], deng[:])
                nc.vector.tensor_tensor(
                    w_st[:, g * NT * K:(g + 1) * NT * K]
                    .rearrange("p (t k) -> p t k", k=K),
                    w8g[:].rearrange("p (t k) -> p t k", k=K),
                    recg[:].unsqueeze(2).broadcast_to((128, NT, K)),
                    op=Alu.mult)

            # ---- store outputs (parallel on the two rings) ----
            if mode == "full":
                if store_mode == "group":
                    for g in range(NG):
                        sl = slice(g * NT * K, (g + 1) * NT * K)
                        nc.sync.dma_start(oidx_d[:, sl],
                                          idx_st[:, sl].bitcast(mybir.dt.int32))
                        nc.scalar.dma_start(ow_d[:, sl], w_st[:, sl])
                else:
                    nc.sync.dma_start(oidx_d, idx_st[:].bitcast(mybir.dt.int32))
                    nc.scalar.dma_start(ow_d, w_st[:])

    nc.compile()
    return nc


def _get_nc():
    if "nc" not in _CACHE:
        _CACHE["nc"] = _build_nc()
    return _CACHE["nc"]


def _pack_x(xm, cpt=4):
    # [T_CORE, H] -> [(g i p), (c t)] tiles; per-partition runs of cpt*GT*2 B
    n_x = KC // cpt
    return np.ascontiguousarray(
        xm.reshape(NG, GT, n_x, cpt, 128)
        .transpose(0, 2, 4, 3, 1)
        .reshape(NG * n_x * 128, cpt * GT)
    )


def _host_prep(hidden_states, weight, expert_biases, cpt=4, lo8=True):
    x = np.asarray(hidden_states, np.float32).reshape(T_TOTAL, H)
    W = np.asarray(weight, np.float32)
    b = np.asarray(expert_biases, np.float32)

    xh = x.astype(np.float16)
    xl_f = (x - xh.astype(np.float32)) * LO_SCALE
    Wh = W.astype(np.float16)
    Wl = ((W - Wh.astype(np.float32)) * LO_SCALE).astype(np.float16)
    if lo8:
        from concourse import mybir
        f8np = mybir.dt.np(mybir.dt.float8e4)
        xl = xl_f.astype(f8np)
        wh8 = None  # built below from Wh
    else:
        xl = xl_f.astype(np.float16)

    # whl[p, k*128 + c]: c<64 -> Wh[c, k*128+p]; c>=64 -> Wl_s[c-64, k*128+p]
    wh_b = np.ascontiguousarray(Wh.T).reshape(KC, 128, E)
    wl_b = np.ascontiguousarray(Wl.T).reshape(KC, 128, E)
    whl = np.concatenate([wh_b, wl_b], axis=2)          # [KC, 128, 128]
    whl = np.ascontiguousarray(
        whl.transpose(1, 0, 2).reshape(128, KC * 2 * E))

    biasb = np.ascontiguousarray(
        np.tile(b.reshape(1, E), (128, NT)).astype(np.float32))

    extra = {}
    if lo8:
        from concourse import mybir
        f8np = mybir.dt.np(mybir.dt.float8e4)
        wh8_b = np.ascontiguousarray(
            Wh.astype(np.float32).T.astype(f8np)).reshape(KC, 128, E)
        extra["wh8"] = np.ascontiguousarray(
            wh8_b.transpose(1, 0, 2).reshape(128, KC * E))

    in_maps = []
    for c in range(N_CORES):
        sl = slice(c * T_CORE, (c + 1) * T_CORE)
        in_maps.append({
            "xht": _pack_x(xh[sl], cpt),
            "xlt": _pack_x(xl[sl], cpt),
            "whl": whl,
            "biasb": biasb,
            **extra,
        })
    return in_maps


def _unpack_out(arr):
    # [128, (g j k)] -> [T_CORE, K]; token = (g*NT + j)*128 + p
    return np.ascontiguousarray(
        arr.reshape(128, NG, NT, K).transpose(1, 2, 0, 3).reshape(T_CORE, K))


def run(hidden_states, weight, expert_biases, trace=False, **spmd_kwargs):
    from concourse.bass_utils import run_bass_kernel_spmd

    in_maps = _host_prep(hidden_states, weight, expert_biases)
    nc = _get_nc()
    res = run_bass_kernel_spmd(
        nc, in_maps, core_ids=list(range(N_CORES)), trace=trace, **spmd_kwargs
    )
    idx = np.concatenate([_unpack_out(r["out_idx"]) for r in res.results],
                         axis=0)
    w = np.concatenate([_unpack_out(r["out_w"]) for r in res.results], axis=0)
    idx = np.ascontiguousarray(idx.reshape(4, 4096, K), dtype=np.int32)
    w = np.ascontiguousarray(w.reshape(4, 4096, K), dtype=np.float32)
    return (idx, w), res


def kernel(**inputs):
    (idx, w), _ = run(**inputs)
    return idx, w


# revision 5
# speedup vs baseline: 1.2750x; 1.2750x over previous
"""MoE gate kernel v2 for Trainium2 (8 NeuronCores, data-parallel over tokens).

Computation per token t (64 experts, top-8):
    gate[t, e]  = sum_h x[t, h] * W[e, h]          (f32-accurate)
    biased      = gate + expert_bias
    top8 of biased -> idx (jax top_k tie semantics)
    weights     = sigmoid(gate[t, idx]) / sum(...)

Precision: hi/lo fp16 split (lo pre-scaled 2^11), as the v1 kernel:
    gate = xh@Wh + 2^-11 * (xh@Wl_s + xl_s@Wh)     (~1e-6 abs err)

v2 layout: x is the STATIONARY operand ([128h x 128t] tiles, FWL-loadable
fp16, dual weight buffers hide the load), W the MOVING operand.  PSUM
comes out [token, expert] directly: no PE transposes, no PSUM->SBUF
transpose copies.  Pass A streams packed [Wh_k | Wl_s_k] (128 cols) over
xh tiles; pass B streams Wh_k (64 cols) over xl tiles accumulating onto
the lo columns.  PE cost 12288 cycles/group vs 16384+transposes in v1.

Top-8 runs on DVE in [t, e] layout: max8/max_index on biased, is_ge mask,
second max8/max_index on masked probs, K x K equality match to reorder
probs into biased-rank order, then L1 normalize.
"""

import numpy as np

N_CORES = 8
H = 2048          # hidden dim = contraction
E = 64            # experts
K = 8             # top-k
T_TOTAL = 16384   # 4*4096 tokens
T_CORE = T_TOTAL // N_CORES   # 2048
NG = 4            # token groups per core
GT = T_CORE // NG             # 512 tokens per group (one PSUM bank of f32)
NT = GT // 128                # 4 token-tiles per group
KC = H // 128                 # 16 contraction chunks
LO_SCALE = float(2.0 ** 11)
INV_LO_SCALE = float(2.0 ** -11)

_CACHE = {}


def _build_nc(repeat=1, mode="full", cpt=8, serialize=False,
              store_mode="group", lo8=True):
    from contextlib import ExitStack

    import concourse.bass as bass  # noqa: F401
    import concourse.tile as tile
    from concourse import bacc, mybir

    f16 = mybir.dt.float16
    f32 = mybir.dt.float32
    u32 = mybir.dt.uint32
    f8 = mybir.dt.float8e4
    xl_dt = f8 if lo8 else f16
    Alu = mybir.AluOpType
    Act = mybir.ActivationFunctionType

    nc = bacc.Bacc(
        "TRN2", target_bir_lowering=False, debug=False, num_devices=N_CORES
    )

    n_x = KC // cpt               # x tiles per group per stream
    rows = cpt // 4               # dram rows per partition per tile

    # DRAM I/O (per core). x shards pre-packed on host (see _host_prep).
    xht_d = nc.dram_tensor("xht", [H, T_CORE], f16, kind="ExternalInput").ap()
    xlt_d = nc.dram_tensor("xlt", [H, T_CORE], xl_dt,
                           kind="ExternalInput").ap()
    whl_d = nc.dram_tensor("whl", [128, KC * 2 * E], f16,
                           kind="ExternalInput").ap()
    if lo8:
        wh8_d = nc.dram_tensor("wh8", [128, KC * E], f8,
                               kind="ExternalInput").ap()
    biasb_d = nc.dram_tensor("biasb", [128, NT * E], f32,
                             kind="ExternalInput").ap()
    oidx_d = nc.dram_tensor("out_idx", [128, NG * NT * K], mybir.dt.int32,
                            kind="ExternalOutput").ap()
    ow_d = nc.dram_tensor("out_w", [128, NG * NT * K], f32,
                          kind="ExternalOutput").ap()
    if mode == "debug":
        dbg_d = nc.dram_tensor("dbg", [128, NG * 3 * NT * 128], f32,
                               kind="ExternalOutput").ap()

    with tile.TileContext(nc) as tc, ExitStack() as ctx:
        xpool = ctx.enter_context(tc.tile_pool(name="x", bufs=1))
        wpool = ctx.enter_context(tc.tile_pool(name="w", bufs=1))
        gpool = ctx.enter_context(tc.tile_pool(name="g", bufs=2))
        ppool = ctx.enter_context(tc.tile_pool(name="mm", bufs=2, space="PSUM"))
        spool = ctx.enter_context(tc.tile_pool(name="s", bufs=2))
        stpool = ctx.enter_context(tc.tile_pool(name="st", bufs=1))

        # weights / bias on the scalar(Act) queue so the sync queue starts
        # streaming xh immediately
        whl = wpool.tile([128, KC * 2 * E], f16, tag="whl")
        nc.scalar.dma_start(whl[:], whl_d)
        biasb = wpool.tile([128, NT * E], f32, tag="biasb")
        nc.scalar.dma_start(biasb[:], biasb_d)
        if lo8:
            wh8 = wpool.tile([128, KC * E], f8, tag="wh8")
            nc.scalar.dma_start(wh8[:], wh8_d)

        idx_st = stpool.tile([128, NG * NT * K], u32, tag="idxst")
        w_st = stpool.tile([128, NG * NT * K], f32, tag="wst")

        if rows == 1:
            src_h = xht_d.rearrange("(b p) f -> b p f", p=128)
            src_l = xlt_d.rearrange("(b p) f -> b p f", p=128)
        else:
            src_h = xht_d.rearrange("(b p r) f -> b p (r f)", p=128, r=rows)
            src_l = xlt_d.rearrange("(b p r) f -> b p (r f)", p=128, r=rows)

        for _rep in range(repeat):
            # ---- x loads, group-major, split across both HWDGE rings ----
            xh_t = [[None] * n_x for _ in range(NG)]
            xl_t = [[None] * n_x for _ in range(NG)]
            for g in range(NG):
                for i in range(n_x):
                    b = g * n_x + i
                    th = xpool.tile([128, cpt * GT], f16, tag=f"xh{g}_{i}")
                    tl = xpool.tile([128, cpt * GT], xl_dt, tag=f"xl{g}_{i}")
                    if serialize and g == 0 and i == 0:
                        # force rep boundary: first x writes on BOTH rings
                        # wait on the previous rep's last top-8 result
                        nc.sync.dma_start(th[0:1, 0:16],
                                          idx_st[0:1, 120:128].bitcast(f16))
                        wsrc = (w_st[0:1, 124:128].bitcast(f8) if lo8
                                else w_st[0:1, 120:128].bitcast(f16))
                        nc.scalar.dma_start(tl[0:1, 0:16], wsrc)
                    nc.sync.dma_start(th[:], src_h[b])
                    nc.scalar.dma_start(tl[:], src_l[b])
                    xh_t[g][i] = th
                    xl_t[g][i] = tl

            def _xs(xt, g, k, j):
                return xt[g][k // cpt][
                    :, (k % cpt) * GT + j * 128:(k % cpt) * GT + (j + 1) * 128]

            for g in range(NG if mode != "dma" else 0):
                # ---- matmuls: psum[t, j*128 + (hi64|lo64)] per tile j ----
                ps = ppool.tile([128, NT * 128], f32, tag="ps")
                # NOTE: start=True zeroes the whole PSUM bank, so exactly one
                # start (first matmul) / stop (last matmul) per bank.
                for k in range(KC):
                    for j in range(NT):
                        nc.tensor.matmul(
                            ps[:, j * 128:(j + 1) * 128],
                            lhsT=_xs(xh_t, g, k, j),
                            rhs=whl[:, k * 2 * E:(k + 1) * 2 * E],
                            start=(k == 0 and j == 0), stop=False)
                for k in range(KC):
                    for j in range(NT):
                        rhs_b = (wh8[:, k * E:(k + 1) * E] if lo8
                                 else whl[:, k * 2 * E:k * 2 * E + E])
                        nc.tensor.matmul(
                            ps[:, j * 128 + E:(j + 1) * 128],
                            lhsT=_xs(xl_t, g, k, j),
                            rhs=rhs_b,
                            start=False,
                            stop=(k == KC - 1 and j == NT - 1))
                if mode == "mm":
                    continue

                psv = ps[:].rearrange("p (j two e) -> p j two e", two=2, e=E)
                if mode == "debug":
                    psb = gpool.tile([128, NT * 128], f32, tag="psb")
                    nc.vector.tensor_copy(psb[:], ps[:])
                    nc.sync.dma_start(
                        dbg_d[:, (g * 3 + 2) * NT * 128:(g * 3 + 3) * NT * 128],
                        psb[:])
                # ---- combine + bias + sigmoid ([t, e] layout) ----
                comb = gpool.tile([128, NT * E], f32, tag="comb")
                combv = comb[:].rearrange("p (j e) -> p j e", e=E)
                nc.scalar.activation(combv, psv[:, :, 1, :], Act.Identity,
                                     scale=INV_LO_SCALE)
                gate = gpool.tile([128, NT * E], f32, tag="gate")
                nc.vector.tensor_tensor(
                    gate[:].rearrange("p (j e) -> p j e", e=E),
                    psv[:, :, 0, :], combv, op=Alu.add)
                probs = gpool.tile([128, NT * E], f32, tag="probs")
                nc.scalar.activation(probs[:], gate[:], Act.Sigmoid)
                biased = gpool.tile([128, NT * E], f32, tag="biased")
                nc.vector.tensor_tensor(biased[:], gate[:], biasb[:],
                                        op=Alu.add)
                if mode == "debug":
                    nc.sync.dma_start(
                        dbg_d[:, (g * 3) * NT * 128:(g * 3) * NT * 128 + NT * E],
                        gate[:])
                    nc.sync.dma_start(
                        dbg_d[:, (g * 3) * NT * 128 + NT * E:
                              (g * 3 + 1) * NT * 128],
                        biased[:])
                    nc.sync.dma_start(
                        dbg_d[:, (g * 3 + 1) * NT * 128:
                              (g * 3 + 1) * NT * 128 + NT * E],
                        probs[:])

                # ---- top-8 (per 128-token tile j; batched where possible) ----
                b8g = spool.tile([128, NT * K], f32, tag="b8g")
                for j in range(NT):
                    nc.vector.max(b8g[:, j * K:(j + 1) * K],
                                  biased[:, j * E:(j + 1) * E])
                    nc.vector.max_index(
                        idx_st[:, (g * NT + j) * K:(g * NT + j + 1) * K],
                        b8g[:, j * K:(j + 1) * K],
                        biased[:, j * E:(j + 1) * E])
                mask = spool.tile([128, NT * E], f32, tag="mask")
                nc.vector.tensor_tensor(
                    mask[:].rearrange("p (t e) -> p t e", e=E),
                    biased[:].rearrange("p (t e) -> p t e", e=E),
                    b8g[:].rearrange("p (t k) -> p t k", k=K)[:, :, 7:8]
                    .broadcast_to((128, NT, E)),
                    op=Alu.is_ge)
                pmask = spool.tile([128, NT * E], f32, tag="pmask")
                nc.vector.tensor_tensor(pmask[:], probs[:], mask[:],
                                        op=Alu.mult)
                p8g = spool.tile([128, NT * K], f32, tag="p8g")
                pidxg = spool.tile([128, NT * K], u32, tag="pidxg")
                for j in range(NT):
                    nc.vector.max(p8g[:, j * K:(j + 1) * K],
                                  pmask[:, j * E:(j + 1) * E])
                    nc.vector.max_index(pidxg[:, j * K:(j + 1) * K],
                                        p8g[:, j * K:(j + 1) * K],
                                        pmask[:, j * E:(j + 1) * E])
                # permute p8 into biased-rank order:
                # w8[t, a] = sum_b p8[t, b] * (pidx[t, b] == bidx[t, a])
                bidx = idx_st[:, g * NT * K:(g + 1) * NT * K]
                eq = spool.tile([128, NT * K * K], f32, tag="eq")
                nc.vector.tensor_tensor(
                    eq[:].rearrange("p (t a b) -> p t a b", a=K, b=K),
                    bidx.rearrange("p (t a) -> p t a", a=K).unsqueeze(3)
                    .broadcast_to((128, NT, K, K)),
                    pidxg[:].rearrange("p (t b) -> p t b", b=K).unsqueeze(2)
                    .broadcast_to((128, NT, K, K)),
                    op=Alu.is_equal)
                wmat = spool.tile([128, NT * K * K], f32, tag="wmat")
                nc.vector.tensor_tensor(
                    wmat[:].rearrange("p (t a b) -> p t a b", a=K, b=K),
                    eq[:].rearrange("p (t a b) -> p t a b", a=K, b=K),
                    p8g[:].rearrange("p (t b) -> p t b", b=K).unsqueeze(2)
                    .broadcast_to((128, NT, K, K)),
                    op=Alu.mult)
                w8g = spool.tile([128, NT * K], f32, tag="w8g")
                nc.vector.tensor_reduce(
                    w8g[:],
                    wmat[:].rearrange("p (ta b) -> p ta b", b=K),
                    axis=mybir.AxisListType.X, op=Alu.add)
                deng = spool.tile([128, NT], f32, tag="deng")
                nc.vector.tensor_reduce(
                    deng[:], w8g[:].rearrange("p (t k) -> p t k", k=K),
                    axis=mybir.AxisListType.X, op=Alu.add)
                recg = spool.tile([128, NT], f32, tag="recg")
                nc.vector.reciprocal(recg[:], deng[:])
                nc.vector.tensor_tensor(
                    w_st[:, g * NT * K:(g + 1) * NT * K]
                    .rearrange("p (t k) -> p t k", k=K),
                    w8g[:].rearrange("p (t k) -> p t k", k=K),
                    recg[:].unsqueeze(2).broadcast_to((128, NT, K)),
                    op=Alu.mult)

            # ---- store outputs (parallel on the two rings) ----
            if mode == "full":
                if store_mode == "group":
                    for g in range(NG):
                        sl = slice(g * NT * K, (g + 1) * NT * K)
                        nc.sync.dma_start(oidx_d[:, sl],
                                          idx_st[:, sl].bitcast(mybir.dt.int32))
                        nc.scalar.dma_start(ow_d[:, sl], w_st[:, sl])
                else:
                    nc.sync.dma_start(oidx_d, idx_st[:].bitcast(mybir.dt.int32))
                    nc.scalar.dma_start(ow_d, w_st[:])

    nc.compile()
    return nc


def _get_nc():
    if "nc" not in _CACHE:
        _CACHE["nc"] = _build_nc()
    return _CACHE["nc"]


def _pack_x(xm, cpt=4):
    # [T_CORE, H] -> [(g i p), (c t)] tiles; per-partition runs of cpt*GT*2 B
    n_x = KC // cpt
    return np.ascontiguousarray(
        xm.reshape(NG, GT, n_x, cpt, 128)
        .transpose(0, 2, 4, 3, 1)
        .reshape(NG * n_x * 128, cpt * GT)
    )


def _host_prep(hidden_states, weight, expert_biases, cpt=8, lo8=True):
    x = np.asarray(hidden_states, np.float32).reshape(T_TOTAL, H)
    W = np.asarray(weight, np.float32)
    b = np.asarray(expert_biases, np.float32)

    xh = x.astype(np.float16)
    xl_f = (x - xh.astype(np.float32)) * LO_SCALE
    Wh = W.astype(np.float16)
    Wl = ((W - Wh.astype(np.float32)) * LO_SCALE).astype(np.float16)
    if lo8:
        from concourse import mybir
        f8np = mybir.dt.np(mybir.dt.float8e4)
        xl = xl_f.astype(f8np)
        wh8 = None  # built below from Wh
    else:
        xl = xl_f.astype(np.float16)

    # whl[p, k*128 + c]: c<64 -> Wh[c, k*128+p]; c>=64 -> Wl_s[c-64, k*128+p]
    wh_b = np.ascontiguousarray(Wh.T).reshape(KC, 128, E)
    wl_b = np.ascontiguousarray(Wl.T).reshape(KC, 128, E)
    whl = np.concatenate([wh_b, wl_b], axis=2)          # [KC, 128, 128]
    whl = np.ascontiguousarray(
        whl.transpose(1, 0, 2).reshape(128, KC * 2 * E))

    biasb = np.ascontiguousarray(
        np.tile(b.reshape(1, E), (128, NT)).astype(np.float32))

    extra = {}
    if lo8:
        from concourse import mybir
        f8np = mybir.dt.np(mybir.dt.float8e4)
        wh8_b = np.ascontiguousarray(
            Wh.astype(np.float32).T.astype(f8np)).reshape(KC, 128, E)
        extra["wh8"] = np.ascontiguousarray(
            wh8_b.transpose(1, 0, 2).reshape(128, KC * E))

    in_maps = []
    for c in range(N_CORES):
        sl = slice(c * T_CORE, (c + 1) * T_CORE)
        in_maps.append({
            "xht": _pack_x(xh[sl], cpt),
            "xlt": _pack_x(xl[sl], cpt),
            "whl": whl,
            "biasb": biasb,
            **extra,
        })
    return in_maps


def _unpack_out(arr):
    # [128, (g j k)] -> [T_CORE, K]; token = (g*NT + j)*128 + p
    return np.ascontiguousarray(
        arr.reshape(128, NG, NT, K).transpose(1, 2, 0, 3).reshape(T_CORE, K))


def run(hidden_states, weight, expert_biases, trace=False, **spmd_kwargs):
    from concourse.bass_utils import run_bass_kernel_spmd

    in_maps = _host_prep(hidden_states, weight, expert_biases)
    nc = _get_nc()
    res = run_bass_kernel_spmd(
        nc, in_maps, core_ids=list(range(N_CORES)), trace=trace, **spmd_kwargs
    )
    idx = np.concatenate([_unpack_out(r["out_idx"]) for r in res.results],
                         axis=0)
    w = np.concatenate([_unpack_out(r["out_w"]) for r in res.results], axis=0)
    idx = np.ascontiguousarray(idx.reshape(4, 4096, K), dtype=np.int32)
    w = np.ascontiguousarray(w.reshape(4, 4096, K), dtype=np.float32)
    return (idx, w), res


def kernel(**inputs):
    (idx, w), _ = run(**inputs)
    return idx, w
